# revision 40
# baseline (speedup 1.0000x reference)
"""Causal self-attention (B=4, T=2048, C=1024, H=16, D=64) on 8 Trainium2 cores.

Sharding: core c = (b, hg) with b = c // 2 (batch), hg = c % 2 (head-group of
8 heads = 512 of 1024 qkv columns). Each core computes q/k/v projections for
its (b, hg), causal attention for its 8 heads, and a partial output
projection y_hg @ Wp[hg]. Host sums the two head-group partials per batch and
adds the projection bias.

Per-core kernel (all matmuls in float32r ~ TF32 precision, softmax in fp32):
interleaved over 512-token quarters: project q/k/v for quarter q, then run
attention for t-block q (its keys/values s <= quarter end are ready) and the
output projection for that t-block. This overlaps the ScalarE-heavy softmax
exp of block q with the PE-heavy projections of quarter q+1.

  - qT/kT [col, t] via lhsT = weight chunk, rhs = xT chunk; v natural [t, col]
    via lhsT = xT chunk, rhs = Wv; v is stored in 65-wide groups per head with
    a ones-column so the attention-value matmul also emits the softmax
    denominator Z (row 64 of the [65, t] PSUM accumulator).
  - scoresT chunks [s=128, t=512] on PE -> exp on ScalarE (no max-subtraction:
    logits are ~N(0,1); fp32 exp cannot overflow) -> causal zeroing of
    block-diagonal chunks via GpSimd affine_select -> [y; Z] accumulation in
    PSUM -> rows scaled by 1/Z -> projection contraction over 512 columns.
"""

import sys

if "/opt/trn_rl_repo" not in sys.path:
    sys.path.insert(0, "/opt/trn_rl_repo")

from contextlib import ExitStack

import numpy as np

import concourse.mybir as mybir
import concourse.tile as tile
from concourse import bacc
from concourse.bass_utils import run_bass_kernel_spmd

F32 = mybir.dt.float32
F32R = mybir.dt.float32r
AF = mybir.ActivationFunctionType

C = 1024      # embed dim
T = 2048      # sequence length
B = 4         # batch
NCOL = 512    # qkv columns per core (8 heads x 64)
TB = 512      # t-block / quarter size
SC = 128      # s-chunk size
D = 64        # head dim

LAST_RESULTS = None  # BassKernelResults of the most recent run (for test.py)
TRACE = False


def _build():
    N_PAIRS = NCOL // 128          # head-pairs per core (4)
    CC = C // 128                  # contraction chunks (8)
    N_TB = T // TB                 # t-blocks / quarters (4)
    SPB = TB // SC                 # s-chunks per t-block (4)
    VGRP = 2 * N_PAIRS             # head groups in v_buf (8)
    VROW = VGRP * 65               # 520

    nc = bacc.Bacc("TRN2", target_bir_lowering=False, debug=False)

    xT = nc.dram_tensor("xT", (C, T), F32R, kind="ExternalInput")
    wq = nc.dram_tensor("wq", (C, NCOL), F32R, kind="ExternalInput")
    wk = nc.dram_tensor("wk", (C, NCOL), F32R, kind="ExternalInput")
    wv = nc.dram_tensor("wv", (C, NCOL), F32R, kind="ExternalInput")
    wp = nc.dram_tensor("wp", (NCOL, C), F32R, kind="ExternalInput")
    bq = nc.dram_tensor("bq", (NCOL, 1), F32, kind="ExternalInput")
    bk = nc.dram_tensor("bk", (NCOL, 1), F32, kind="ExternalInput")
    bv = nc.dram_tensor("bv", (1, NCOL), F32R, kind="ExternalInput")
    out = nc.dram_tensor("out", (T, C), F32, kind="ExternalOutput")

    with tile.TileContext(nc) as tc, ExitStack() as ctx:
        const = ctx.enter_context(tc.tile_pool(name="const", bufs=1))
        xq_pool = ctx.enter_context(tc.tile_pool(name="xq", bufs=2))
        w_pool = ctx.enter_context(tc.tile_pool(name="wqkv", bufs=1))
        qt_pool = ctx.enter_context(tc.tile_pool(name="qt", bufs=2))
        att_pool = ctx.enter_context(tc.tile_pool(name="att", bufs=2))
        yt_pool = ctx.enter_context(tc.tile_pool(name="yt", bufs=1))
        small = ctx.enter_context(tc.tile_pool(name="small", bufs=1))
        ostage = ctx.enter_context(tc.tile_pool(name="ostage", bufs=2))
        ps_acc = ctx.enter_context(tc.tile_pool(name="ps_acc", bufs=2, space="PSUM"))
        ps1 = ps_acc
        ps_po = ps_acc
        ps_sc = ctx.enter_context(tc.tile_pool(name="ps_sc", bufs=2, space="PSUM"))
        ps_yz = ctx.enter_context(tc.tile_pool(name="ps_yz", bufs=2, space="PSUM"))

        kT = const.tile([128, N_PAIRS * T], F32R, tag="kT")   # [col_in_pair, p*T + s]
        v_buf = const.tile([128, (T // SC) * VROW], F32R, tag="vbuf")
        wp_sb = const.tile([128, N_PAIRS * C], F32R, tag="wp")
        bq_sb = const.tile([128, N_PAIRS], F32, tag="bq")
        bk_sb = const.tile([128, N_PAIRS], F32, tag="bk")
        bv_sb = const.tile([1, NCOL], F32R, tag="bv")
        ones_sb = const.tile([1, 128], F32R, tag="ones")
        wq_sb = w_pool.tile([128, CC * NCOL], F32R, tag="wq")
        wk_sb = w_pool.tile([128, CC * NCOL], F32R, tag="wk")
        wv_sb = w_pool.tile([128, CC * NCOL], F32R, tag="wv")

        # startup DMAs chunk-by-chunk (x chunk, then this chunk of each
        # weight) so the first projection matmuls start as soon as possible.
        xh_tiles = {}
        xh_tiles[0] = xq_pool.tile([128, CC * TB], F32R, tag="xh", name="xh0")
        # one queue, priority order: the v-units unblock first (xh+wv), then
        # q, then k; serial per-queue DMAs each run at full HBM bandwidth
        # startup inputs strictly serialized on the ACT queue in priority
        # order (each runs at full HBM bandwidth); the sync queue stays free
        # for x prefetches and output stores
        nc.sync.dma_start(
            xh_tiles[0][:].rearrange("a (cc t) -> a cc t", cc=CC),
            xT.ap()[:, 0:TB].rearrange("(cc a) t -> a cc t", a=128),
        )
        nc.scalar.dma_start(
            wv_sb[:].rearrange("a (cc n) -> a cc n", cc=CC),
            wv.ap().rearrange("(cc a) n -> a cc n", a=128),
        )
        nc.sync.dma_start(
            wq_sb[:].rearrange("a (cc n) -> a cc n", cc=CC),
            wq.ap().rearrange("(cc a) n -> a cc n", a=128),
        )
        nc.gpsimd.dma_start(
            wk_sb[:].rearrange("a (cc n) -> a cc n", cc=CC),
            wk.ap().rearrange("(cc a) n -> a cc n", a=128),
        )
        nc.sync.dma_start(
            wp_sb[:].rearrange("a (p n) -> a p n", p=N_PAIRS),
            wp.ap().rearrange("(p a) n -> a p n", a=128),
        )
        nc.sync.dma_start(
            bq_sb[:][:, :, None], bq.ap().rearrange("(p a) o -> a p o", a=128)
        )
        nc.sync.dma_start(
            bk_sb[:][:, :, None], bk.ap().rearrange("(p a) o -> a p o", a=128)
        )
        # 0/1 causal triangle mask: msk[s, f] = (f >= s); every block-diagonal
        # offset r uses the width-(TB - r*SC) prefix of the same tile
        msk = const.tile([128, TB], F32R, tag="msk")
        msk_f32 = ostage.tile([128, 512], F32, tag="ob", name="msk_f32")
        nc.gpsimd.memset(msk_f32[:], 1.0)
        nc.gpsimd.affine_select(
            out=msk_f32[:],
            in_=msk_f32[:],
            compare_op=mybir.AluOpType.is_ge,
            fill=0.0,
            base=0,
            channel_multiplier=-1,
            pattern=[[1, TB]],
        )
        nc.vector.tensor_copy(msk[:], msk_f32[:])
        ones_f32 = const.tile([128, max(128, (T // SC) * VGRP)], F32, tag="ones_f32")
        nc.vector.memset(ones_f32[:], 1.0)
        nc.vector.tensor_copy(ones_sb[:], ones_f32[0:1, 0:128])
        nc.sync.dma_start(bv_sb[:], bv.ap())
        # ones columns of v_buf (col 64 of each 65-group)
        nc.vector.tensor_copy(
            v_buf[:].rearrange("a (t g o) -> a t g o", g=VGRP, o=65)[:, :, :, 64:65],
            ones_f32[:, : (T // SC) * VGRP].rearrange("a (t g) -> a t g", g=VGRP)[
                :, :, :, None
            ],
        )

        def emit_qkv_unit(tb, u):
            """Unit u of quarter tb: 0..2*N_PAIRS-1 = (pair, q|k) groups,
            then TB//128 v-groups."""
            t0 = tb * TB
            xh = xh_tiles[tb]
            if u < 2 * N_PAIRS:
                p, which = u // 2, u % 2
                wt, bias = ((wq_sb, bq_sb), (wk_sb, bk_sb))[which]
                dst = (
                    qt_tiles[tb][:, p * TB : (p + 1) * TB]
                    if which == 0
                    else kT[:, p * T + t0 : p * T + t0 + TB]
                )
                pt = ps1.tile([128, TB], F32, tag="acc")
                for cc in range(CC):
                    nc.tensor.matmul(
                        pt[:],
                        wt[:, cc * NCOL + p * 128 : cc * NCOL + p * 128 + 128],
                        xh[:, cc * TB : cc * TB + TB],
                        start=(cc == 0),
                        stop=(cc == CC - 1),
                    )
                nc.vector.tensor_scalar_add(dst, pt[:], bias[:, p : p + 1])
            else:
                tth = u - 2 * N_PAIRS
                tt = (t0 // 128) + tth
                pt = ps1.tile([128, NCOL], F32, tag="acc")
                for cc in range(CC):
                    nc.tensor.matmul(
                        pt[:],
                        xh[:, cc * TB + tth * 128 : cc * TB + tth * 128 + 128],
                        wv_sb[:, cc * NCOL : (cc + 1) * NCOL],
                        start=(cc == 0),
                        stop=False,
                    )
                nc.tensor.matmul(
                    pt[:], ones_sb[:, 0:128], bv_sb[:], start=False, stop=True
                )
                nc.vector.tensor_copy(
                    v_buf[:, tt * VROW : (tt + 1) * VROW].rearrange(
                        "a (g o) -> a g o", g=VGRP
                    )[:, :, 0:64],
                    pt[:].rearrange("a (g o) -> a g o", g=VGRP),
                )

        def att_head(tb, p, h, fill=None):
            hrow = h * 64
            qT = qt_tiles[tb]
            yt = yt_tiles[tb]
            yz = ps_yz.tile([128, TB], F32, tag="yz")
            n_chunk = SPB * tb + SPB
            # diagonal chunks first: their exp->affine_select mask chain then
            # overlaps with the plain chunks' matmuls instead of stalling AV
            if tb > 0:
                # first pair plain (fast start=True AV), then diagonal chunks
                # (their mask chain overlaps later plain chunks), then the rest
                j_order = (
                    [0, 1]
                    + list(range(SPB * tb, n_chunk))
                    + list(range(2, SPB * tb))
                )
            else:
                j_order = list(range(n_chunk))
            for jj in range(0, n_chunk, 2):
                st = ps_sc.tile([128, 2 * TB], F32, tag="st")
                at = att_pool.tile([128, 2 * TB], F32R, tag="at")
                cols = []
                for k in range(2):
                    j = j_order[jj + k]
                    r = j - SPB * tb  # >=0 only for block-diag chunks
                    c0 = max(0, r * SC)  # first valid t-col
                    o = k * TB
                    cols.append((j, r, c0, o))
                    nc.tensor.matmul(
                        st[:, o + c0 : o + TB],
                        kT[hrow : hrow + 64, p * T + j * SC : p * T + j * SC + SC],
                        qT[hrow : hrow + 64, p * TB + c0 : (p + 1) * TB],
                        start=True,
                        stop=True,
                    )
                if cols[0][1] < 0 and cols[1][1] < 0:
                    # both fully causal: one batched exp over both chunks
                    nc.scalar.activation(at[:, 0 : 2 * TB], st[:, 0 : 2 * TB], AF.Exp)
                else:
                    for j, r, c0, o in cols:
                        nc.scalar.activation(
                            at[:, o + c0 : o + TB], st[:, o + c0 : o + TB], AF.Exp
                        )
                for kk, (j, r, c0, o) in enumerate(cols):
                    if r >= 0:
                        # zero att where t_loc < r*SC + s_loc (multiply by the
                        # precomputed 0/1 diag mask; cheaper chain than Pool)
                        nc.vector.tensor_mul(
                            at[:, o + c0 : o + TB],
                            at[:, o + c0 : o + TB],
                            msk[:, 0 : TB - c0],
                        )
                    vj = v_buf[
                        :,
                        j * VROW + (2 * p + h) * 65 : j * VROW + (2 * p + h) * 65 + 65,
                    ]
                    nc.tensor.matmul(
                        yz[0:65, c0:TB],
                        vj,
                        at[:, o + c0 : o + TB],
                        start=(jj + kk == 0),
                        stop=(jj + kk == n_chunk - 1),
                    )
                    if fill is not None:
                        fill(1)
            rz = small.tile([1, TB], F32, tag="rz")
            nc.vector.reciprocal(rz[:], yz[64:65, :])
            rzb = small.tile([64, TB], F32, tag="rzb")
            nc.gpsimd.partition_broadcast(rzb[:], rz[:])
            nc.vector.tensor_mul(
                yt[hrow : hrow + 64, p * TB : (p + 1) * TB],
                yz[0:64, :],
                rzb[:],
            )

        N_UNITS = 2 * N_PAIRS + TB // 128  # 12
        qt_tiles = {}
        yt_tiles = {}
        qt_tiles[0] = qt_pool.tile([128, N_PAIRS * TB], F32R, tag="qT", name="qT0")
        for u in list(range(2 * N_PAIRS, N_UNITS)) + list(range(2 * N_PAIRS)):
            emit_qkv_unit(0, u)

        def qkv_thunks(tb):
            """Per-matmul thunks for quarter tb's projections, to be spliced
            one-at-a-time into the attention stream of quarter tb-1."""
            thunks = []
            t0 = tb * TB
            xh = xh_tiles[tb]
            for u in range(2 * N_PAIRS):
                p, which = u // 2, u % 2
                wt, bias = ((wq_sb, bq_sb), (wk_sb, bk_sb))[which]
                dst = (
                    qt_tiles[tb][:, p * TB : (p + 1) * TB]
                    if which == 0
                    else kT[:, p * T + t0 : p * T + t0 + TB]
                )
                pt_box = [None]
                def mk(cc, u=u, p=p, wt=wt, bias=bias, dst=dst, pt_box=pt_box):
                    def go():
                        if cc == 0:
                            pt_box[0] = ps1.tile([128, TB], F32, tag="acc", name=f"ps_{tb}_{u}")
                        pt = pt_box[0]
                        nc.tensor.matmul(
                            pt[:],
                            wt[:, cc * NCOL + p * 128 : cc * NCOL + p * 128 + 128],
                            xh[:, cc * TB : cc * TB + TB],
                            start=(cc == 0),
                            stop=(cc == CC - 1),
                        )
                        if cc == CC - 1:
                            nc.vector.tensor_scalar_add(dst, pt[:], bias[:, p : p + 1])
                    return go
                thunks.extend(mk(cc) for cc in range(CC))
            for tth in range(TB // 128):
                tt = (t0 // 128) + tth
                pt_box = [None]
                def mkv(cc, tth=tth, tt=tt, pt_box=pt_box):
                    def go():
                        if cc == 0:
                            pt_box[0] = ps1.tile([128, NCOL], F32, tag="acc", name=f"psv_{tb}_{tth}")
                        pt = pt_box[0]
                        if cc < CC:
                            nc.tensor.matmul(
                                pt[:],
                                xh[:, cc * TB + tth * 128 : cc * TB + tth * 128 + 128],
                                wv_sb[:, cc * NCOL : (cc + 1) * NCOL],
                                start=(cc == 0),
                                stop=False,
                            )
                        else:
                            nc.tensor.matmul(
                                pt[:], ones_sb[:, 0:128], bv_sb[:], start=False, stop=True
                            )
                            nc.vector.tensor_copy(
                                v_buf[:, tt * VROW : (tt + 1) * VROW].rearrange(
                                    "a (g o) -> a g o", g=VGRP
                                )[:, :, 0:64],
                                pt[:].rearrange("a (g o) -> a g o", g=VGRP),
                            )
                    return go
                thunks.extend(mkv(cc) for cc in range(CC + 1))
            return thunks

        def proj_thunks(tb):
            """Per-matmul thunks for t-block tb's output projection."""
            t0 = tb * TB
            yt = yt_tiles[tb]
            thunks = []
            for tt in range(TB // 128):
                for nh in range(C // 512):
                    po_box = [None]
                    def mk(p, tt=tt, nh=nh, po_box=po_box):
                        def go():
                            if p == 0:
                                po_box[0] = ps_po.tile(
                                    [128, 512], F32, tag="acc",
                                    name=f"po_{tb}_{tt}_{nh}",
                                )
                            po = po_box[0]
                            nc.tensor.matmul(
                                po[:],
                                yt[:, p * TB + tt * 128 : p * TB + tt * 128 + 128],
                                wp_sb[:, p * C + nh * 512 : p * C + nh * 512 + 512],
                                start=(p == 0),
                                stop=(p == N_PAIRS - 1),
                            )
                            if p == N_PAIRS - 1:
                                ob = ostage.tile([128, 512], F32, tag="ob")
                                nc.vector.tensor_copy(ob[:], po[:])
                                nc.sync.dma_start(
                                    out.ap()[
                                        t0 + tt * 128 : t0 + tt * 128 + 128,
                                        nh * 512 : (nh + 1) * 512,
                                    ],
                                    ob[:],
                                )
                        return go
                    thunks.extend(mk(p) for p in range(N_PAIRS))
            return thunks

        for tb in range(N_TB):
            t0 = tb * TB
            # prefetch next quarter's x
            thunks = []
            if tb + 1 < N_TB:
                nxt = xq_pool.tile([128, CC * TB], F32R, tag="xh", name=f"xh{tb+1}")
                xh_tiles[tb + 1] = nxt
                nc.sync.dma_start(
                    nxt[:].rearrange("a (cc t) -> a cc t", cc=CC),
                    xT.ap()[:, t0 + TB : t0 + 2 * TB].rearrange(
                        "(cc a) t -> a cc t", a=128
                    ),
                )
                qt_tiles[tb + 1] = qt_pool.tile(
                    [128, N_PAIRS * TB], F32R, tag="qT", name=f"qT{tb+1}"
                )
                thunks = qkv_thunks(tb + 1)
            if tb == N_TB - 1:
                thunks = thunks + proj_thunks(tb - 1)
            yt_tiles[tb] = yt_pool.tile([128, N_PAIRS * TB], F32R, tag="yt", name=f"yt{tb}")

            # attention chunks with next quarter's projection matmuls spliced
            # in one per chunk slot, keeping PE busy while ScalarE runs exp
            n_slots = 8 * (SPB * tb + SPB)
            slot = [0]
            def fill(k):
                lo = slot[0] * len(thunks) // n_slots
                slot[0] = min(slot[0] + k, n_slots)
                hi = slot[0] * len(thunks) // n_slots
                for th in thunks[lo:hi]:
                    th()
            heads = [(p, h) for p in range(N_PAIRS) for h in range(2)]
            for p, h in heads:
                att_head(tb, p, h, fill)
            fill(n_slots)  # any remainder
            xh_tiles.pop(tb)
            if tb < N_TB - 2:
                for th in proj_thunks(tb):
                    th()

        # final t-block's projection
        for th in proj_thunks(N_TB - 1):
            th()

    nc.compile()
    return nc


_NC_CACHE = None


def kernel(x, Wq, bq, Wk, bk, Wv, bv, Wp, bp):
    global LAST_RESULTS, _NC_CACHE
    x = np.asarray(x, dtype=np.float32)
    Wq = np.asarray(Wq, dtype=np.float32)
    Wk = np.asarray(Wk, dtype=np.float32)
    Wv = np.asarray(Wv, dtype=np.float32)
    Wp = np.asarray(Wp, dtype=np.float32)
    bq = np.asarray(bq, dtype=np.float32)
    bk = np.asarray(bk, dtype=np.float32)
    bv = np.asarray(bv, dtype=np.float32)
    bp = np.asarray(bp, dtype=np.float32)

    if _NC_CACHE is None:
        _NC_CACHE = _build()
    nc = _NC_CACHE

    scale = 1.0 / np.sqrt(D)
    in_maps = []
    for core in range(8):
        b, hg = core // 2, core % 2
        cols = slice(hg * NCOL, (hg + 1) * NCOL)
        in_maps.append(
            {
                "xT": np.ascontiguousarray(x[b].T),
                "wq": np.ascontiguousarray(Wq[:, cols]) * scale,
                "wk": np.ascontiguousarray(Wk[:, cols]),
                "wv": np.ascontiguousarray(Wv[:, cols]),
                "wp": np.ascontiguousarray(Wp[cols, :]),
                "bq": (bq[cols] * scale).reshape(NCOL, 1).copy(),
                "bk": bk[cols].reshape(NCOL, 1).copy(),
                "bv": bv[cols].reshape(1, NCOL).copy(),
            }
        )

    res = run_bass_kernel_spmd(nc, in_maps, core_ids=list(range(8)), trace=TRACE)
    LAST_RESULTS = res

    result = np.empty((B, T, C), dtype=np.float32)
    for b in range(B):
        result[b] = res.results[2 * b]["out"] + res.results[2 * b + 1]["out"] + bp
    return result


# revision 45
# speedup vs baseline: 1.0126x; 1.0126x over previous
"""Causal self-attention (B=4, T=2048, C=1024, H=16, D=64) on 8 Trainium2 cores.

Sharding: core c = (b, hg) with b = c // 2 (batch), hg = c % 2 (head-group of
8 heads = 512 of 1024 qkv columns). Each core computes q/k/v projections for
its (b, hg), causal attention for its 8 heads, and a partial output
projection y_hg @ Wp[hg]. Host sums the two head-group partials per batch and
adds the projection bias.

Per-core kernel (all matmuls in float32r ~ TF32 precision, softmax in fp32):
interleaved over 512-token quarters: project q/k/v for quarter q, then run
attention for t-block q (its keys/values s <= quarter end are ready) and the
output projection for that t-block. This overlaps the ScalarE-heavy softmax
exp of block q with the PE-heavy projections of quarter q+1.

  - qT/kT [col, t] via lhsT = weight chunk, rhs = xT chunk; v natural [t, col]
    via lhsT = xT chunk, rhs = Wv; v is stored in 65-wide groups per head with
    a ones-column so the attention-value matmul also emits the softmax
    denominator Z (row 64 of the [65, t] PSUM accumulator).
  - scoresT chunks [s=128, t=512] on PE -> exp on ScalarE (no max-subtraction:
    logits are ~N(0,1); fp32 exp cannot overflow) -> causal zeroing of
    block-diagonal chunks via GpSimd affine_select -> [y; Z] accumulation in
    PSUM -> rows scaled by 1/Z -> projection contraction over 512 columns.
"""

import sys

if "/opt/trn_rl_repo" not in sys.path:
    sys.path.insert(0, "/opt/trn_rl_repo")

from contextlib import ExitStack

import numpy as np

import concourse.mybir as mybir
import concourse.tile as tile
from concourse import bacc
from concourse.bass_utils import run_bass_kernel_spmd

F32 = mybir.dt.float32
F32R = mybir.dt.float32r
AF = mybir.ActivationFunctionType

C = 1024      # embed dim
T = 2048      # sequence length
B = 4         # batch
NCOL = 512    # qkv columns per core (8 heads x 64)
TB = 512      # t-block / quarter size
SC = 128      # s-chunk size
D = 64        # head dim

LAST_RESULTS = None  # BassKernelResults of the most recent run (for test.py)
TRACE = False


def _build():
    N_PAIRS = NCOL // 128          # head-pairs per core (4)
    CC = C // 128                  # contraction chunks (8)
    N_TB = T // TB                 # t-blocks / quarters (4)
    SPB = TB // SC                 # s-chunks per t-block (4)
    VGRP = 2 * N_PAIRS             # head groups in v_buf (8)
    VROW = VGRP * 65               # 520

    nc = bacc.Bacc("TRN2", target_bir_lowering=False, debug=False)

    xT = nc.dram_tensor("xT", (C, T), F32R, kind="ExternalInput")
    wq = nc.dram_tensor("wq", (C, NCOL), F32R, kind="ExternalInput")
    wk = nc.dram_tensor("wk", (C, NCOL), F32R, kind="ExternalInput")
    wv = nc.dram_tensor("wv", (C, NCOL), F32R, kind="ExternalInput")
    wp = nc.dram_tensor("wp", (NCOL, C), F32R, kind="ExternalInput")
    bq = nc.dram_tensor("bq", (NCOL, 1), F32, kind="ExternalInput")
    bk = nc.dram_tensor("bk", (NCOL, 1), F32, kind="ExternalInput")
    bv = nc.dram_tensor("bv", (1, NCOL), F32R, kind="ExternalInput")
    out = nc.dram_tensor("out", (T, C), F32, kind="ExternalOutput")

    with tile.TileContext(nc) as tc, ExitStack() as ctx:
        const = ctx.enter_context(tc.tile_pool(name="const", bufs=1))
        xq_pool = ctx.enter_context(tc.tile_pool(name="xq", bufs=2))
        w_pool = ctx.enter_context(tc.tile_pool(name="wqkv", bufs=1))
        qt_pool = ctx.enter_context(tc.tile_pool(name="qt", bufs=2))
        att_pool = ctx.enter_context(tc.tile_pool(name="att", bufs=2))
        yt_pool = ctx.enter_context(tc.tile_pool(name="yt", bufs=1))
        small = ctx.enter_context(tc.tile_pool(name="small", bufs=1))
        ostage = ctx.enter_context(tc.tile_pool(name="ostage", bufs=2))
        ps_acc = ctx.enter_context(tc.tile_pool(name="ps_acc", bufs=2, space="PSUM"))
        ps1 = ps_acc
        ps_po = ps_acc
        ps_sc = ctx.enter_context(tc.tile_pool(name="ps_sc", bufs=2, space="PSUM"))
        ps_yz = ctx.enter_context(tc.tile_pool(name="ps_yz", bufs=2, space="PSUM"))

        kT = const.tile([128, N_PAIRS * T], F32R, tag="kT")   # [col_in_pair, p*T + s]
        v_buf = const.tile([128, (T // SC) * VROW], F32R, tag="vbuf")
        wp_sb = const.tile([128, N_PAIRS * C], F32R, tag="wp")
        bq_sb = const.tile([128, N_PAIRS], F32, tag="bq")
        bk_sb = const.tile([128, N_PAIRS], F32, tag="bk")
        bv_sb = const.tile([1, NCOL], F32R, tag="bv")
        ones_sb = const.tile([1, 128], F32R, tag="ones")
        wq_sb = w_pool.tile([128, CC * NCOL], F32R, tag="wq")
        wk_sb = w_pool.tile([128, CC * NCOL], F32R, tag="wk")
        wv_sb = w_pool.tile([128, CC * NCOL], F32R, tag="wv")

        # startup DMAs chunk-by-chunk (x chunk, then this chunk of each
        # weight) so the first projection matmuls start as soon as possible.
        xh_tiles = {}
        xh_tiles[0] = xq_pool.tile([128, CC * TB], F32R, tag="xh", name="xh0")
        # one queue, priority order: the v-units unblock first (xh+wv), then
        # q, then k; serial per-queue DMAs each run at full HBM bandwidth
        # startup inputs strictly serialized on the ACT queue in priority
        # order (each runs at full HBM bandwidth); the sync queue stays free
        # for x prefetches and output stores
        nc.sync.dma_start(
            xh_tiles[0][:].rearrange("a (cc t) -> a cc t", cc=CC),
            xT.ap()[:, 0:TB].rearrange("(cc a) t -> a cc t", a=128),
        )
        nc.scalar.dma_start(
            wv_sb[:].rearrange("a (cc n) -> a cc n", cc=CC),
            wv.ap().rearrange("(cc a) n -> a cc n", a=128),
        )
        nc.sync.dma_start(
            wq_sb[:].rearrange("a (cc n) -> a cc n", cc=CC),
            wq.ap().rearrange("(cc a) n -> a cc n", a=128),
        )
        nc.gpsimd.dma_start(
            wk_sb[:].rearrange("a (cc n) -> a cc n", cc=CC),
            wk.ap().rearrange("(cc a) n -> a cc n", a=128),
        )
        nc.sync.dma_start(
            wp_sb[:].rearrange("a (p n) -> a p n", p=N_PAIRS),
            wp.ap().rearrange("(p a) n -> a p n", a=128),
        )
        nc.sync.dma_start(
            bq_sb[:][:, :, None], bq.ap().rearrange("(p a) o -> a p o", a=128)
        )
        nc.sync.dma_start(
            bk_sb[:][:, :, None], bk.ap().rearrange("(p a) o -> a p o", a=128)
        )
        # 0/1 causal triangle mask: msk[s, f] = (f >= s); every block-diagonal
        # offset r uses the width-(TB - r*SC) prefix of the same tile
        msk = const.tile([128, TB], F32R, tag="msk")
        msk_f32 = ostage.tile([128, 512], F32, tag="ob", name="msk_f32")
        nc.gpsimd.memset(msk_f32[:], 1.0)
        nc.gpsimd.affine_select(
            out=msk_f32[:],
            in_=msk_f32[:],
            compare_op=mybir.AluOpType.is_ge,
            fill=0.0,
            base=0,
            channel_multiplier=-1,
            pattern=[[1, TB]],
        )
        nc.vector.tensor_copy(msk[:], msk_f32[:])
        # offset-SC triangle for the widened r=3 chunks: keep iff f >= s + SC
        msk3 = const.tile([128, 2 * SC], F32R, tag="msk3")
        nc.gpsimd.memset(msk_f32[:, 0 : 2 * SC], 1.0)
        nc.gpsimd.affine_select(
            out=msk_f32[:, 0 : 2 * SC],
            in_=msk_f32[:, 0 : 2 * SC],
            compare_op=mybir.AluOpType.is_ge,
            fill=0.0,
            base=-SC,
            channel_multiplier=-1,
            pattern=[[1, 2 * SC]],
        )
        nc.vector.tensor_copy(msk3[:], msk_f32[:, 0 : 2 * SC])
        ones_f32 = const.tile([128, max(128, (T // SC) * VGRP)], F32, tag="ones_f32")
        nc.vector.memset(ones_f32[:], 1.0)
        nc.vector.tensor_copy(ones_sb[:], ones_f32[0:1, 0:128])
        nc.sync.dma_start(bv_sb[:], bv.ap())
        # ones columns of v_buf (col 64 of each 65-group)
        nc.vector.tensor_copy(
            v_buf[:].rearrange("a (t g o) -> a t g o", g=VGRP, o=65)[:, :, :, 64:65],
            ones_f32[:, : (T // SC) * VGRP].rearrange("a (t g) -> a t g", g=VGRP)[
                :, :, :, None
            ],
        )

        def emit_qkv_unit(tb, u):
            """Unit u of quarter tb: 0..2*N_PAIRS-1 = (pair, q|k) groups,
            then TB//128 v-groups."""
            t0 = tb * TB
            xh = xh_tiles[tb]
            if u < 2 * N_PAIRS:
                p, which = u // 2, u % 2
                wt, bias = ((wq_sb, bq_sb), (wk_sb, bk_sb))[which]
                dst = (
                    qt_tiles[tb][:, p * TB : (p + 1) * TB]
                    if which == 0
                    else kT[:, p * T + t0 : p * T + t0 + TB]
                )
                pt = ps1.tile([128, TB], F32, tag="acc")
                for cc in range(CC):
                    nc.tensor.matmul(
                        pt[:],
                        wt[:, cc * NCOL + p * 128 : cc * NCOL + p * 128 + 128],
                        xh[:, cc * TB : cc * TB + TB],
                        start=(cc == 0),
                        stop=(cc == CC - 1),
                    )
                nc.vector.tensor_scalar_add(dst, pt[:], bias[:, p : p + 1])
            else:
                tth = u - 2 * N_PAIRS
                tt = (t0 // 128) + tth
                pt = ps1.tile([128, NCOL], F32, tag="acc")
                for cc in range(CC):
                    nc.tensor.matmul(
                        pt[:],
                        xh[:, cc * TB + tth * 128 : cc * TB + tth * 128 + 128],
                        wv_sb[:, cc * NCOL : (cc + 1) * NCOL],
                        start=(cc == 0),
                        stop=False,
                    )
                nc.tensor.matmul(
                    pt[:], ones_sb[:, 0:128], bv_sb[:], start=False, stop=True
                )
                nc.vector.tensor_copy(
                    v_buf[:, tt * VROW : (tt + 1) * VROW].rearrange(
                        "a (g o) -> a g o", g=VGRP
                    )[:, :, 0:64],
                    pt[:].rearrange("a (g o) -> a g o", g=VGRP),
                )

        def att_head(tb, p, h, fill=None):
            hrow = h * 64
            qT = qt_tiles[tb]
            yt = yt_tiles[tb]
            yz = ps_yz.tile([128, TB], F32, tag="yz")
            n_chunk = SPB * tb + SPB
            # diagonal chunks first: their exp->affine_select mask chain then
            # overlaps with the plain chunks' matmuls instead of stalling AV
            if tb > 0:
                # first pair plain (fast start=True AV), then diagonal chunks
                # (their mask chain overlaps later plain chunks), then the rest
                j_order = (
                    [0, 1]
                    + list(range(SPB * tb, n_chunk))
                    + list(range(2, SPB * tb))
                )
            else:
                j_order = list(range(n_chunk))
            for jj in range(0, n_chunk, 2):
                st = ps_sc.tile([128, 2 * TB], F32, tag="st")
                at = att_pool.tile([128, 2 * TB], F32R, tag="at")
                cols = []
                for k in range(2):
                    j = j_order[jj + k]
                    r = j - SPB * tb  # >=0 only for block-diag chunks
                    c0 = max(0, r * SC)  # first valid t-col
                    # widen N=128 slices to 256: fp32r runs 4 cyc/row below
                    # N=256, so the wider matmul is 2x faster; the extra
                    # columns are zeroed by the offset mask
                    c0 = min(c0, TB - 2 * SC)
                    o = k * TB
                    cols.append((j, r, c0, o))
                    nc.tensor.matmul(
                        st[:, o + c0 : o + TB],
                        kT[hrow : hrow + 64, p * T + j * SC : p * T + j * SC + SC],
                        qT[hrow : hrow + 64, p * TB + c0 : (p + 1) * TB],
                        start=True,
                        stop=True,
                    )
                if cols[0][1] < 0 and cols[1][1] < 0:
                    # both fully causal: one batched exp over both chunks
                    nc.scalar.activation(at[:, 0 : 2 * TB], st[:, 0 : 2 * TB], AF.Exp)
                else:
                    for j, r, c0, o in cols:
                        nc.scalar.activation(
                            at[:, o + c0 : o + TB], st[:, o + c0 : o + TB], AF.Exp
                        )
                for kk, (j, r, c0, o) in enumerate(cols):
                    if r >= 0:
                        # zero att where t_loc < r*SC + s_loc (multiply by the
                        # precomputed 0/1 diag mask; cheaper chain than Pool)
                        m = msk3 if r * SC > c0 else msk
                        nc.vector.tensor_mul(
                            at[:, o + c0 : o + TB],
                            at[:, o + c0 : o + TB],
                            m[:, 0 : TB - c0],
                        )
                    vj = v_buf[
                        :,
                        j * VROW + (2 * p + h) * 65 : j * VROW + (2 * p + h) * 65 + 65,
                    ]
                    nc.tensor.matmul(
                        yz[0:65, c0:TB],
                        vj,
                        at[:, o + c0 : o + TB],
                        start=(jj + kk == 0),
                        stop=(jj + kk == n_chunk - 1),
                    )
                    if fill is not None:
                        fill(1)
            rz = small.tile([1, TB], F32, tag="rz")
            nc.vector.reciprocal(rz[:], yz[64:65, :])
            rzb = small.tile([64, TB], F32, tag="rzb")
            nc.gpsimd.partition_broadcast(rzb[:], rz[:])
            nc.vector.tensor_mul(
                yt[hrow : hrow + 64, p * TB : (p + 1) * TB],
                yz[0:64, :],
                rzb[:],
            )

        N_UNITS = 2 * N_PAIRS + TB // 128  # 12
        qt_tiles = {}
        yt_tiles = {}
        qt_tiles[0] = qt_pool.tile([128, N_PAIRS * TB], F32R, tag="qT", name="qT0")
        for u in list(range(2 * N_PAIRS, N_UNITS)) + list(range(2 * N_PAIRS)):
            emit_qkv_unit(0, u)

        def qkv_thunks(tb):
            """Per-matmul thunks for quarter tb's projections, to be spliced
            one-at-a-time into the attention stream of quarter tb-1."""
            thunks = []
            t0 = tb * TB
            xh = xh_tiles[tb]
            for u in range(2 * N_PAIRS):
                p, which = u // 2, u % 2
                wt, bias = ((wq_sb, bq_sb), (wk_sb, bk_sb))[which]
                dst = (
                    qt_tiles[tb][:, p * TB : (p + 1) * TB]
                    if which == 0
                    else kT[:, p * T + t0 : p * T + t0 + TB]
                )
                pt_box = [None]
                def mk(cc, u=u, p=p, wt=wt, bias=bias, dst=dst, pt_box=pt_box):
                    def go():
                        if cc == 0:
                            pt_box[0] = ps1.tile([128, TB], F32, tag="acc", name=f"ps_{tb}_{u}")
                        pt = pt_box[0]
                        nc.tensor.matmul(
                            pt[:],
                            wt[:, cc * NCOL + p * 128 : cc * NCOL + p * 128 + 128],
                            xh[:, cc * TB : cc * TB + TB],
                            start=(cc == 0),
                            stop=(cc == CC - 1),
                        )
                        if cc == CC - 1:
                            nc.vector.tensor_scalar_add(dst, pt[:], bias[:, p : p + 1])
                    return go
                thunks.extend(mk(cc) for cc in range(CC))
            for tth in range(TB // 128):
                tt = (t0 // 128) + tth
                pt_box = [None]
                def mkv(cc, tth=tth, tt=tt, pt_box=pt_box):
                    def go():
                        if cc == 0:
                            pt_box[0] = ps1.tile([128, NCOL], F32, tag="acc", name=f"psv_{tb}_{tth}")
                        pt = pt_box[0]
                        if cc < CC:
                            nc.tensor.matmul(
                                pt[:],
                                xh[:, cc * TB + tth * 128 : cc * TB + tth * 128 + 128],
                                wv_sb[:, cc * NCOL : (cc + 1) * NCOL],
                                start=(cc == 0),
                                stop=False,
                            )
                        else:
                            nc.tensor.matmul(
                                pt[:], ones_sb[:, 0:128], bv_sb[:], start=False, stop=True
                            )
                            nc.vector.tensor_copy(
                                v_buf[:, tt * VROW : (tt + 1) * VROW].rearrange(
                                    "a (g o) -> a g o", g=VGRP
                                )[:, :, 0:64],
                                pt[:].rearrange("a (g o) -> a g o", g=VGRP),
                            )
                    return go
                thunks.extend(mkv(cc) for cc in range(CC + 1))
            return thunks

        def proj_thunks(tb):
            """Per-matmul thunks for t-block tb's output projection."""
            t0 = tb * TB
            yt = yt_tiles[tb]
            thunks = []
            for tt in range(TB // 128):
                for nh in range(C // 512):
                    po_box = [None]
                    def mk(p, tt=tt, nh=nh, po_box=po_box):
                        def go():
                            if p == 0:
                                po_box[0] = ps_po.tile(
                                    [128, 512], F32, tag="acc",
                                    name=f"po_{tb}_{tt}_{nh}",
                                )
                            po = po_box[0]
                            nc.tensor.matmul(
                                po[:],
                                yt[:, p * TB + tt * 128 : p * TB + tt * 128 + 128],
                                wp_sb[:, p * C + nh * 512 : p * C + nh * 512 + 512],
                                start=(p == 0),
                                stop=(p == N_PAIRS - 1),
                            )
                            if p == N_PAIRS - 1:
                                ob = ostage.tile([128, 512], F32, tag="ob")
                                nc.vector.tensor_copy(ob[:], po[:])
                                nc.sync.dma_start(
                                    out.ap()[
                                        t0 + tt * 128 : t0 + tt * 128 + 128,
                                        nh * 512 : (nh + 1) * 512,
                                    ],
                                    ob[:],
                                )
                        return go
                    thunks.extend(mk(p) for p in range(N_PAIRS))
            return thunks

        for tb in range(N_TB):
            t0 = tb * TB
            # prefetch next quarter's x
            thunks = []
            if tb + 1 < N_TB:
                nxt = xq_pool.tile([128, CC * TB], F32R, tag="xh", name=f"xh{tb+1}")
                xh_tiles[tb + 1] = nxt
                nc.sync.dma_start(
                    nxt[:].rearrange("a (cc t) -> a cc t", cc=CC),
                    xT.ap()[:, t0 + TB : t0 + 2 * TB].rearrange(
                        "(cc a) t -> a cc t", a=128
                    ),
                )
                qt_tiles[tb + 1] = qt_pool.tile(
                    [128, N_PAIRS * TB], F32R, tag="qT", name=f"qT{tb+1}"
                )
                thunks = qkv_thunks(tb + 1)
            if tb == N_TB - 1:
                thunks = thunks + proj_thunks(tb - 1)
            yt_tiles[tb] = yt_pool.tile([128, N_PAIRS * TB], F32R, tag="yt", name=f"yt{tb}")

            # attention chunks with next quarter's projection matmuls spliced
            # in one per chunk slot, keeping PE busy while ScalarE runs exp
            n_slots = 8 * (SPB * tb + SPB)
            slot = [0]
            def fill(k):
                lo = slot[0] * len(thunks) // n_slots
                slot[0] = min(slot[0] + k, n_slots)
                hi = slot[0] * len(thunks) // n_slots
                for th in thunks[lo:hi]:
                    th()
            heads = [(p, h) for p in range(N_PAIRS) for h in range(2)]
            for p, h in heads:
                att_head(tb, p, h, fill)
            fill(n_slots)  # any remainder
            xh_tiles.pop(tb)
            if tb < N_TB - 2:
                for th in proj_thunks(tb):
                    th()

        # final t-block's projection
        for th in proj_thunks(N_TB - 1):
            th()

    nc.compile()
    return nc


_NC_CACHE = None


def kernel(x, Wq, bq, Wk, bk, Wv, bv, Wp, bp):
    global LAST_RESULTS, _NC_CACHE
    x = np.asarray(x, dtype=np.float32)
    Wq = np.asarray(Wq, dtype=np.float32)
    Wk = np.asarray(Wk, dtype=np.float32)
    Wv = np.asarray(Wv, dtype=np.float32)
    Wp = np.asarray(Wp, dtype=np.float32)
    bq = np.asarray(bq, dtype=np.float32)
    bk = np.asarray(bk, dtype=np.float32)
    bv = np.asarray(bv, dtype=np.float32)
    bp = np.asarray(bp, dtype=np.float32)

    if _NC_CACHE is None:
        _NC_CACHE = _build()
    nc = _NC_CACHE

    scale = 1.0 / np.sqrt(D)
    in_maps = []
    for core in range(8):
        b, hg = core // 2, core % 2
        cols = slice(hg * NCOL, (hg + 1) * NCOL)
        in_maps.append(
            {
                "xT": np.ascontiguousarray(x[b].T),
                "wq": np.ascontiguousarray(Wq[:, cols]) * scale,
                "wk": np.ascontiguousarray(Wk[:, cols]),
                "wv": np.ascontiguousarray(Wv[:, cols]),
                "wp": np.ascontiguousarray(Wp[cols, :]),
                "bq": (bq[cols] * scale).reshape(NCOL, 1).copy(),
                "bk": bk[cols].reshape(NCOL, 1).copy(),
                "bv": bv[cols].reshape(1, NCOL).copy(),
            }
        )

    res = run_bass_kernel_spmd(nc, in_maps, core_ids=list(range(8)), trace=TRACE)
    LAST_RESULTS = res

    result = np.empty((B, T, C), dtype=np.float32)
    for b in range(B):
        result[b] = res.results[2 * b]["out"] + res.results[2 * b + 1]["out"] + bp
    return result


# revision 51
# speedup vs baseline: 1.0142x; 1.0016x over previous
"""Causal self-attention (B=4, T=2048, C=1024, H=16, D=64) on 8 Trainium2 cores.

Sharding: core c = (b, hg) with b = c // 2 (batch), hg = c % 2 (head-group of
8 heads = 512 of 1024 qkv columns). Each core computes q/k/v projections for
its (b, hg), causal attention for its 8 heads, and a partial output
projection y_hg @ Wp[hg]. Host sums the two head-group partials per batch and
adds the projection bias.

Per-core kernel (all matmuls in float32r ~ TF32 precision, softmax in fp32):
interleaved over 512-token quarters: project q/k/v for quarter q, then run
attention for t-block q (its keys/values s <= quarter end are ready) and the
output projection for that t-block. This overlaps the ScalarE-heavy softmax
exp of block q with the PE-heavy projections of quarter q+1.

  - qT/kT [col, t] via lhsT = weight chunk, rhs = xT chunk; v natural [t, col]
    via lhsT = xT chunk, rhs = Wv; v is stored in 65-wide groups per head with
    a ones-column so the attention-value matmul also emits the softmax
    denominator Z (row 64 of the [65, t] PSUM accumulator).
  - scoresT chunks [s=128, t=512] on PE -> exp on ScalarE (no max-subtraction:
    logits are ~N(0,1); fp32 exp cannot overflow) -> causal zeroing of
    block-diagonal chunks via GpSimd affine_select -> [y; Z] accumulation in
    PSUM -> rows scaled by 1/Z -> projection contraction over 512 columns.
"""

import sys

if "/opt/trn_rl_repo" not in sys.path:
    sys.path.insert(0, "/opt/trn_rl_repo")

from contextlib import ExitStack

import numpy as np

import concourse.mybir as mybir
import concourse.tile as tile
from concourse import bacc
from concourse.bass_utils import run_bass_kernel_spmd

F32 = mybir.dt.float32
F32R = mybir.dt.float32r
AF = mybir.ActivationFunctionType

C = 1024      # embed dim
T = 2048      # sequence length
B = 4         # batch
NCOL = 512    # qkv columns per core (8 heads x 64)
TB = 512      # t-block / quarter size
SC = 128      # s-chunk size
D = 64        # head dim

LAST_RESULTS = None  # BassKernelResults of the most recent run (for test.py)
TRACE = False


def _build():
    N_PAIRS = NCOL // 128          # head-pairs per core (4)
    CC = C // 128                  # contraction chunks (8)
    N_TB = T // TB                 # t-blocks / quarters (4)
    SPB = TB // SC                 # s-chunks per t-block (4)
    VGRP = 2 * N_PAIRS             # head groups in v_buf (8)
    VROW = VGRP * 65               # 520

    nc = bacc.Bacc("TRN2", target_bir_lowering=False, debug=False)

    xT = nc.dram_tensor("xT", (C, T), F32R, kind="ExternalInput")
    wq = nc.dram_tensor("wq", (C, NCOL), F32R, kind="ExternalInput")
    wk = nc.dram_tensor("wk", (C, NCOL), F32R, kind="ExternalInput")
    wv = nc.dram_tensor("wv", (C, NCOL), F32R, kind="ExternalInput")
    wp = nc.dram_tensor("wp", (NCOL, C), F32R, kind="ExternalInput")
    bq = nc.dram_tensor("bq", (NCOL, 1), F32, kind="ExternalInput")
    bk = nc.dram_tensor("bk", (NCOL, 1), F32, kind="ExternalInput")
    bv = nc.dram_tensor("bv", (1, NCOL), F32R, kind="ExternalInput")
    out = nc.dram_tensor("out", (T, C), F32, kind="ExternalOutput")

    with tile.TileContext(nc) as tc, ExitStack() as ctx:
        const = ctx.enter_context(tc.tile_pool(name="const", bufs=1))
        xq_pool = ctx.enter_context(tc.tile_pool(name="xq", bufs=2))
        w_pool = ctx.enter_context(tc.tile_pool(name="wqkv", bufs=1))
        qt_pool = ctx.enter_context(tc.tile_pool(name="qt", bufs=2))
        att_pool = ctx.enter_context(tc.tile_pool(name="att", bufs=2))
        yt_pool = ctx.enter_context(tc.tile_pool(name="yt", bufs=1))
        small = ctx.enter_context(tc.tile_pool(name="small", bufs=1))
        ostage = ctx.enter_context(tc.tile_pool(name="ostage", bufs=2))
        ps_acc = ctx.enter_context(tc.tile_pool(name="ps_acc", bufs=2, space="PSUM"))
        ps1 = ps_acc
        ps_po = ps_acc
        ps_sc = ctx.enter_context(tc.tile_pool(name="ps_sc", bufs=2, space="PSUM"))
        ps_yz = ctx.enter_context(tc.tile_pool(name="ps_yz", bufs=2, space="PSUM"))

        kT = const.tile([128, N_PAIRS * T], F32R, tag="kT")   # [col_in_pair, p*T + s]
        v_buf = const.tile([128, (T // SC) * VROW], F32R, tag="vbuf")
        wp_sb = const.tile([128, N_PAIRS * C], F32R, tag="wp")
        bq_sb = const.tile([128, N_PAIRS], F32, tag="bq")
        bk_sb = const.tile([128, N_PAIRS], F32, tag="bk")
        bv_sb = const.tile([1, NCOL], F32R, tag="bv")
        ones_sb = const.tile([1, 128], F32R, tag="ones")
        wq_sb = w_pool.tile([128, CC * NCOL], F32R, tag="wq")
        wk_sb = w_pool.tile([128, CC * NCOL], F32R, tag="wk")
        wv_sb = w_pool.tile([128, CC * NCOL], F32R, tag="wv")

        # startup DMAs chunk-by-chunk (x chunk, then this chunk of each
        # weight) so the first projection matmuls start as soon as possible.
        xh_tiles = {}
        xh_tiles[0] = xq_pool.tile([128, CC * TB], F32R, tag="xh", name="xh0")
        # one queue, priority order: the v-units unblock first (xh+wv), then
        # q, then k; serial per-queue DMAs each run at full HBM bandwidth
        # startup inputs strictly serialized on the ACT queue in priority
        # order (each runs at full HBM bandwidth); the sync queue stays free
        # for x prefetches and output stores
        nc.sync.dma_start(
            xh_tiles[0][:].rearrange("a (cc t) -> a cc t", cc=CC),
            xT.ap()[:, 0:TB].rearrange("(cc a) t -> a cc t", a=128),
        )
        nc.scalar.dma_start(
            wv_sb[:].rearrange("a (cc n) -> a cc n", cc=CC),
            wv.ap().rearrange("(cc a) n -> a cc n", a=128),
        )
        nc.sync.dma_start(
            wq_sb[:].rearrange("a (cc n) -> a cc n", cc=CC),
            wq.ap().rearrange("(cc a) n -> a cc n", a=128),
        )
        nc.gpsimd.dma_start(
            wk_sb[:].rearrange("a (cc n) -> a cc n", cc=CC),
            wk.ap().rearrange("(cc a) n -> a cc n", a=128),
        )
        nc.sync.dma_start(
            wp_sb[:].rearrange("a (p n) -> a p n", p=N_PAIRS),
            wp.ap().rearrange("(p a) n -> a p n", a=128),
        )
        nc.sync.dma_start(
            bq_sb[:][:, :, None], bq.ap().rearrange("(p a) o -> a p o", a=128)
        )
        nc.sync.dma_start(
            bk_sb[:][:, :, None], bk.ap().rearrange("(p a) o -> a p o", a=128)
        )
        # 0/1 causal triangle mask: msk[s, f] = (f >= s); every block-diagonal
        # offset r uses the width-(TB - r*SC) prefix of the same tile
        msk = const.tile([128, TB], F32R, tag="msk")
        msk_f32 = ostage.tile([128, 512], F32, tag="ob", name="msk_f32")
        nc.gpsimd.memset(msk_f32[:], 1.0)
        nc.gpsimd.affine_select(
            out=msk_f32[:],
            in_=msk_f32[:],
            compare_op=mybir.AluOpType.is_ge,
            fill=0.0,
            base=0,
            channel_multiplier=-1,
            pattern=[[1, TB]],
        )
        nc.vector.tensor_copy(msk[:], msk_f32[:])
        # offset-SC triangle for the widened r=3 chunks: keep iff f >= s + SC
        msk3 = const.tile([128, 2 * SC], F32R, tag="msk3")
        nc.gpsimd.memset(msk_f32[:, 0 : 2 * SC], 1.0)
        nc.gpsimd.affine_select(
            out=msk_f32[:, 0 : 2 * SC],
            in_=msk_f32[:, 0 : 2 * SC],
            compare_op=mybir.AluOpType.is_ge,
            fill=0.0,
            base=-SC,
            channel_multiplier=-1,
            pattern=[[1, 2 * SC]],
        )
        nc.vector.tensor_copy(msk3[:], msk_f32[:, 0 : 2 * SC])
        # PE warm-up: dummy matmuls on the DMA-independent mask tile keep the
        # PE clock ramped while the input DMAs stream; a guard read into an
        # unused cell keeps them alive through DCE
        warm_ps = ps_sc.tile([128, 2 * TB], F32, tag="st", name="warm_ps")
        for _ in range(28):
            nc.tensor.matmul(
                warm_ps[:, 0:TB], msk[:, 0:128], msk[:], start=True, stop=True
            )
        guard = const.tile([1, 1], F32, tag="guard")
        nc.vector.tensor_copy(guard[:], warm_ps[0:1, 0:1])
        nc.sync.dma_start(out.ap()[0:1, 0:1], guard[:])
        ones_f32 = const.tile([128, max(128, (T // SC) * VGRP)], F32, tag="ones_f32")
        nc.vector.memset(ones_f32[:], 1.0)
        nc.vector.tensor_copy(ones_sb[:], ones_f32[0:1, 0:128])
        nc.sync.dma_start(bv_sb[:], bv.ap())
        # ones columns of v_buf (col 64 of each 65-group)
        nc.vector.tensor_copy(
            v_buf[:].rearrange("a (t g o) -> a t g o", g=VGRP, o=65)[:, :, :, 64:65],
            ones_f32[:, : (T // SC) * VGRP].rearrange("a (t g) -> a t g", g=VGRP)[
                :, :, :, None
            ],
        )

        def emit_qkv_unit(tb, u):
            """Unit u of quarter tb: 0..2*N_PAIRS-1 = (pair, q|k) groups,
            then TB//128 v-groups."""
            t0 = tb * TB
            xh = xh_tiles[tb]
            if u < 2 * N_PAIRS:
                p, which = u // 2, u % 2
                wt, bias = ((wq_sb, bq_sb), (wk_sb, bk_sb))[which]
                dst = (
                    qt_tiles[tb][:, p * TB : (p + 1) * TB]
                    if which == 0
                    else kT[:, p * T + t0 : p * T + t0 + TB]
                )
                pt = ps1.tile([128, TB], F32, tag="acc")
                for cc in range(CC):
                    nc.tensor.matmul(
                        pt[:],
                        wt[:, cc * NCOL + p * 128 : cc * NCOL + p * 128 + 128],
                        xh[:, cc * TB : cc * TB + TB],
                        start=(cc == 0),
                        stop=(cc == CC - 1),
                    )
                nc.vector.tensor_scalar_add(dst, pt[:], bias[:, p : p + 1])
            else:
                tth = u - 2 * N_PAIRS
                tt = (t0 // 128) + tth
                pt = ps1.tile([128, NCOL], F32, tag="acc")
                for cc in range(CC):
                    nc.tensor.matmul(
                        pt[:],
                        xh[:, cc * TB + tth * 128 : cc * TB + tth * 128 + 128],
                        wv_sb[:, cc * NCOL : (cc + 1) * NCOL],
                        start=(cc == 0),
                        stop=False,
                    )
                nc.tensor.matmul(
                    pt[:], ones_sb[:, 0:128], bv_sb[:], start=False, stop=True
                )
                nc.vector.tensor_copy(
                    v_buf[:, tt * VROW : (tt + 1) * VROW].rearrange(
                        "a (g o) -> a g o", g=VGRP
                    )[:, :, 0:64],
                    pt[:].rearrange("a (g o) -> a g o", g=VGRP),
                )

        def att_head(tb, p, h, fill=None):
            hrow = h * 64
            qT = qt_tiles[tb]
            yt = yt_tiles[tb]
            yz = ps_yz.tile([128, TB], F32, tag="yz")
            n_chunk = SPB * tb + SPB
            # diagonal chunks first: their exp->affine_select mask chain then
            # overlaps with the plain chunks' matmuls instead of stalling AV
            if tb > 0:
                # first pair plain (fast start=True AV), then diagonal chunks
                # (their mask chain overlaps later plain chunks), then the rest
                j_order = (
                    [0, 1]
                    + list(range(SPB * tb, n_chunk))
                    + list(range(2, SPB * tb))
                )
            else:
                j_order = list(range(n_chunk))
            for jj in range(0, n_chunk, 2):
                st = ps_sc.tile([128, 2 * TB], F32, tag="st")
                at = att_pool.tile([128, 2 * TB], F32R, tag="at")
                cols = []
                for k in range(2):
                    j = j_order[jj + k]
                    r = j - SPB * tb  # >=0 only for block-diag chunks
                    c0 = max(0, r * SC)  # first valid t-col
                    # widen N=128 slices to 256: fp32r runs 4 cyc/row below
                    # N=256, so the wider matmul is 2x faster; the extra
                    # columns are zeroed by the offset mask
                    c0 = min(c0, TB - 2 * SC)
                    o = k * TB
                    cols.append((j, r, c0, o))
                    nc.tensor.matmul(
                        st[:, o + c0 : o + TB],
                        kT[hrow : hrow + 64, p * T + j * SC : p * T + j * SC + SC],
                        qT[hrow : hrow + 64, p * TB + c0 : (p + 1) * TB],
                        start=True,
                        stop=True,
                    )
                if cols[0][1] < 0 and cols[1][1] < 0:
                    # both fully causal: one batched exp over both chunks
                    nc.scalar.activation(at[:, 0 : 2 * TB], st[:, 0 : 2 * TB], AF.Exp)
                else:
                    for j, r, c0, o in cols:
                        nc.scalar.activation(
                            at[:, o + c0 : o + TB], st[:, o + c0 : o + TB], AF.Exp
                        )
                for kk, (j, r, c0, o) in enumerate(cols):
                    if r >= 0:
                        # zero att where t_loc < r*SC + s_loc (multiply by the
                        # precomputed 0/1 diag mask; cheaper chain than Pool)
                        m = msk3 if r * SC > c0 else msk
                        nc.vector.tensor_mul(
                            at[:, o + c0 : o + TB],
                            at[:, o + c0 : o + TB],
                            m[:, 0 : TB - c0],
                        )
                    vj = v_buf[
                        :,
                        j * VROW + (2 * p + h) * 65 : j * VROW + (2 * p + h) * 65 + 65,
                    ]
                    nc.tensor.matmul(
                        yz[0:65, c0:TB],
                        vj,
                        at[:, o + c0 : o + TB],
                        start=(jj + kk == 0),
                        stop=(jj + kk == n_chunk - 1),
                    )
                    if fill is not None:
                        fill(1)
            rz = small.tile([1, TB], F32, tag="rz")
            nc.vector.reciprocal(rz[:], yz[64:65, :])
            rzb = small.tile([64, TB], F32, tag="rzb")
            nc.gpsimd.partition_broadcast(rzb[:], rz[:])
            nc.vector.tensor_mul(
                yt[hrow : hrow + 64, p * TB : (p + 1) * TB],
                yz[0:64, :],
                rzb[:],
            )

        N_UNITS = 2 * N_PAIRS + TB // 128  # 12
        qt_tiles = {}
        yt_tiles = {}
        qt_tiles[0] = qt_pool.tile([128, N_PAIRS * TB], F32R, tag="qT", name="qT0")
        for u in list(range(2 * N_PAIRS, N_UNITS)) + list(range(2 * N_PAIRS)):
            emit_qkv_unit(0, u)

        def qkv_thunks(tb):
            """Per-matmul thunks for quarter tb's projections, to be spliced
            one-at-a-time into the attention stream of quarter tb-1."""
            thunks = []
            t0 = tb * TB
            xh = xh_tiles[tb]
            for u in range(2 * N_PAIRS):
                p, which = u // 2, u % 2
                wt, bias = ((wq_sb, bq_sb), (wk_sb, bk_sb))[which]
                dst = (
                    qt_tiles[tb][:, p * TB : (p + 1) * TB]
                    if which == 0
                    else kT[:, p * T + t0 : p * T + t0 + TB]
                )
                pt_box = [None]
                def mk(cc, u=u, p=p, wt=wt, bias=bias, dst=dst, pt_box=pt_box):
                    def go():
                        if cc == 0:
                            pt_box[0] = ps1.tile([128, TB], F32, tag="acc", name=f"ps_{tb}_{u}")
                        pt = pt_box[0]
                        nc.tensor.matmul(
                            pt[:],
                            wt[:, cc * NCOL + p * 128 : cc * NCOL + p * 128 + 128],
                            xh[:, cc * TB : cc * TB + TB],
                            start=(cc == 0),
                            stop=(cc == CC - 1),
                        )
                        if cc == CC - 1:
                            nc.vector.tensor_scalar_add(dst, pt[:], bias[:, p : p + 1])
                    return go
                thunks.extend(mk(cc) for cc in range(CC))
            for tth in range(TB // 128):
                tt = (t0 // 128) + tth
                pt_box = [None]
                def mkv(cc, tth=tth, tt=tt, pt_box=pt_box):
                    def go():
                        if cc == 0:
                            pt_box[0] = ps1.tile([128, NCOL], F32, tag="acc", name=f"psv_{tb}_{tth}")
                        pt = pt_box[0]
                        if cc < CC:
                            nc.tensor.matmul(
                                pt[:],
                                xh[:, cc * TB + tth * 128 : cc * TB + tth * 128 + 128],
                                wv_sb[:, cc * NCOL : (cc + 1) * NCOL],
                                start=(cc == 0),
                                stop=False,
                            )
                        else:
                            nc.tensor.matmul(
                                pt[:], ones_sb[:, 0:128], bv_sb[:], start=False, stop=True
                            )
                            nc.vector.tensor_copy(
                                v_buf[:, tt * VROW : (tt + 1) * VROW].rearrange(
                                    "a (g o) -> a g o", g=VGRP
                                )[:, :, 0:64],
                                pt[:].rearrange("a (g o) -> a g o", g=VGRP),
                            )
                    return go
                thunks.extend(mkv(cc) for cc in range(CC + 1))
            return thunks

        def proj_thunks(tb):
            """Per-matmul thunks for t-block tb's output projection."""
            t0 = tb * TB
            yt = yt_tiles[tb]
            thunks = []
            for tt in range(TB // 128):
                for nh in range(C // 512):
                    po_box = [None]
                    def mk(p, tt=tt, nh=nh, po_box=po_box):
                        def go():
                            if p == 0:
                                po_box[0] = ps_po.tile(
                                    [128, 512], F32, tag="acc",
                                    name=f"po_{tb}_{tt}_{nh}",
                                )
                            po = po_box[0]
                            nc.tensor.matmul(
                                po[:],
                                yt[:, p * TB + tt * 128 : p * TB + tt * 128 + 128],
                                wp_sb[:, p * C + nh * 512 : p * C + nh * 512 + 512],
                                start=(p == 0),
                                stop=(p == N_PAIRS - 1),
                            )
                            if p == N_PAIRS - 1:
                                ob = ostage.tile([128, 512], F32, tag="ob")
                                nc.vector.tensor_copy(ob[:], po[:])
                                nc.sync.dma_start(
                                    out.ap()[
                                        t0 + tt * 128 : t0 + tt * 128 + 128,
                                        nh * 512 : (nh + 1) * 512,
                                    ],
                                    ob[:],
                                )
                        return go
                    thunks.extend(mk(p) for p in range(N_PAIRS))
            return thunks

        for tb in range(N_TB):
            t0 = tb * TB
            # prefetch next quarter's x
            thunks = []
            if tb + 1 < N_TB:
                nxt = xq_pool.tile([128, CC * TB], F32R, tag="xh", name=f"xh{tb+1}")
                xh_tiles[tb + 1] = nxt
                nc.sync.dma_start(
                    nxt[:].rearrange("a (cc t) -> a cc t", cc=CC),
                    xT.ap()[:, t0 + TB : t0 + 2 * TB].rearrange(
                        "(cc a) t -> a cc t", a=128
                    ),
                )
                qt_tiles[tb + 1] = qt_pool.tile(
                    [128, N_PAIRS * TB], F32R, tag="qT", name=f"qT{tb+1}"
                )
                thunks = qkv_thunks(tb + 1)
            if tb == N_TB - 1:
                thunks = thunks + proj_thunks(tb - 1)
            yt_tiles[tb] = yt_pool.tile([128, N_PAIRS * TB], F32R, tag="yt", name=f"yt{tb}")

            # attention chunks with next quarter's projection matmuls spliced
            # in one per chunk slot, keeping PE busy while ScalarE runs exp
            n_slots = 8 * (SPB * tb + SPB)
            slot = [0]
            def fill(k):
                lo = slot[0] * len(thunks) // n_slots
                slot[0] = min(slot[0] + k, n_slots)
                hi = slot[0] * len(thunks) // n_slots
                for th in thunks[lo:hi]:
                    th()
            heads = [(p, h) for p in range(N_PAIRS) for h in range(2)]
            for p, h in heads:
                att_head(tb, p, h, fill)
            fill(n_slots)  # any remainder
            xh_tiles.pop(tb)
            if tb < N_TB - 2:
                for th in proj_thunks(tb):
                    th()

        # final t-block's projection
        for th in proj_thunks(N_TB - 1):
            th()

    nc.compile()
    return nc


_NC_CACHE = None


def kernel(x, Wq, bq, Wk, bk, Wv, bv, Wp, bp):
    global LAST_RESULTS, _NC_CACHE
    x = np.asarray(x, dtype=np.float32)
    Wq = np.asarray(Wq, dtype=np.float32)
    Wk = np.asarray(Wk, dtype=np.float32)
    Wv = np.asarray(Wv, dtype=np.float32)
    Wp = np.asarray(Wp, dtype=np.float32)
    bq = np.asarray(bq, dtype=np.float32)
    bk = np.asarray(bk, dtype=np.float32)
    bv = np.asarray(bv, dtype=np.float32)
    bp = np.asarray(bp, dtype=np.float32)

    if _NC_CACHE is None:
        _NC_CACHE = _build()
    nc = _NC_CACHE

    scale = 1.0 / np.sqrt(D)
    in_maps = []
    for core in range(8):
        b, hg = core // 2, core % 2
        cols = slice(hg * NCOL, (hg + 1) * NCOL)
        in_maps.append(
            {
                "xT": np.ascontiguousarray(x[b].T),
                "wq": np.ascontiguousarray(Wq[:, cols]) * scale,
                "wk": np.ascontiguousarray(Wk[:, cols]),
                "wv": np.ascontiguousarray(Wv[:, cols]),
                "wp": np.ascontiguousarray(Wp[cols, :]),
                "bq": (bq[cols] * scale).reshape(NCOL, 1).copy(),
                "bk": bk[cols].reshape(NCOL, 1).copy(),
                "bv": bv[cols].reshape(1, NCOL).copy(),
            }
        )

    res = run_bass_kernel_spmd(nc, in_maps, core_ids=list(range(8)), trace=TRACE)
    LAST_RESULTS = res

    result = np.empty((B, T, C), dtype=np.float32)
    for b in range(B):
        result[b] = res.results[2 * b]["out"] + res.results[2 * b + 1]["out"] + bp
    return result
